# revision 33
# baseline (speedup 1.0000x reference)
"""Trainium2 Bass kernel for Adapt_CSA (ECA channel attention + CBAM spatial attention).

Reference computation (per sample, x: (C=256, H=64, W=64) f32):
  y    = mean(x, (H,W))                       # (C,)
  y'   = conv1d(y, w_c, SAME, k=5)            # (C,)
  yc   = sigmoid(x * y'[:, None, None])       # (C, H, W)
  avg  = mean(yc, C); mx = max(yc, C)         # (H, W) each
  sa   = sigmoid(conv2d([avg, mx], w_s, SAME))# (1, H, W)
  out  = yc * sa + x

Data parallel over batch: 32 samples -> 8 cores x 4 samples.
Layout on chip: channels on partitions (2 tiles of (128, 4096) per sample).
x is uploaded in bf16 (rel-err budget is 2e-2); output stored bf16 and
widened to f32 on the host.

Emission is software-pipelined in 4 skewed stages per sample --
H: load + global-avg-pool, E: channel conv1d + sigmoid, M: spatial branch,
T: multiply + residual + store -- so that every engine's (in-order)
instruction stream is phase-sorted and sample s+1's head never queues
behind sample s's tail on the same engine.

The 5x5 conv uses "68-pitch" patches: avg/max maps are written into
zero-padded 68x68 DRAM images; each of the 50 patch rows (c, ky, kx) is one
contiguous 4348-element window of the padded image, so the conv becomes a
single (50 x 4352) matmul against the 50 folded weights and the output is
pixel-indexed with pitch 68 (columns 64..67 of each row are garbage).
"""

import os
import sys

import numpy as np

sys.path.insert(0, "/opt/trn_rl_repo")

B, C, H, W = 32, 256, 64, 64
HW = H * W  # 4096
N_CORES = 8
SPC = B // N_CORES  # samples per core = 4
PAD = 68  # 64 + 2*2 zero border for SAME 5x5 conv
PJ = 16 * PAD  # 1088: 68-pitch pixel range per 16-row block
PJ4 = 4 * PJ  # 4352

_cache = {}


def _build_graph():
    import concourse.bass as bass
    import concourse.bacc as bacc
    import concourse.tile as tile
    from concourse import masks, mybir

    f32 = mybir.dt.float32
    bf16 = mybir.dt.bfloat16
    AF = mybir.ActivationFunctionType
    ALU = mybir.AluOpType
    AX = mybir.AxisListType

    nc = bacc.Bacc("TRN2", target_bir_lowering=False)

    x_ext = nc.declare_dram_parameter("x", [SPC, 2, 128, HW], bf16, isOutput=False)
    bmat_ext = nc.declare_dram_parameter("bmat", [2, 128, 256], f32, isOutput=False)
    ws_ext = nc.declare_dram_parameter("ws", [50, 1], f32, isOutput=False)
    out_ext = nc.declare_dram_parameter("out", [SPC, 2, 128, HW], bf16, isOutput=True)

    # double-buffered padded avg/max maps in DRAM: [buf][c][ph][pw]
    pads = nc.dram_tensor("pads", [2, 2, PAD, PAD], bf16)
    # double-buffered sa row (pixel-linear) in DRAM for partition broadcast
    sa_dram = nc.dram_tensor("sa_dram", [2, HW], bf16)

    with tile.TileContext(nc) as tc:
        with (
            tc.tile_pool(name="singles", bufs=1) as singles,
            tc.tile_pool(name="px", bufs=4) as px,
            tc.tile_pool(name="pyc", bufs=4) as pyc,
            tc.tile_pool(name="ppm", bufs=2) as ppm,
            tc.tile_pool(name="ppatch", bufs=2) as ppatch,
            tc.tile_pool(name="psab", bufs=2) as psab,
            tc.tile_pool(name="small", bufs=4) as small,
            tc.tile_pool(name="ps_y", bufs=1, space="PSUM") as ps_y,
            tc.tile_pool(name="ps_t", bufs=2, space="PSUM") as ps_t,
            tc.tile_pool(name="ps_mean", bufs=1, space="PSUM") as ps_mean,
            tc.tile_pool(name="ps_sa", bufs=1, space="PSUM") as ps_sa,
        ):
            # ---- constants ----
            bmat_sb = singles.tile([128, 2, 256], f32)
            nc.sync.dma_start(out=bmat_sb, in_=bmat_ext[:].rearrange("t p m -> p t m"))
            ws_f32 = singles.tile([50, 1], f32)
            nc.sync.dma_start(out=ws_f32, in_=ws_ext[:])
            ws_bf = singles.tile([50, 1], bf16)
            nc.vector.tensor_copy(out=ws_bf, in_=ws_f32)
            ident = singles.tile([128, 128], bf16)
            masks.make_identity(nc, ident[:])
            ones_bf = singles.tile([128, 1], bf16)
            nc.vector.memset(ones_bf, 1.0)
            gapjunk = singles.tile([128, HW], bf16)
            zero68 = singles.tile([PAD, 2, PAD], bf16)
            nc.vector.memset(zero68, 0.0)
            for j in range(2):
                nc.sync.dma_start(
                    out=pads[j].rearrange("c h w -> h c w"), in_=zero68
                )

            # per-sample state carried between stages
            st = [dict() for _ in range(SPC)]

            def stage_H(s):
                x_t = px.tile([128, 2, HW], bf16)
                if s < 2:
                    # prologue: split loads across engines/queues for latency
                    nc.sync.dma_start(out=x_t[:, 0, 0:2048], in_=x_ext[s, 0, :, 0:2048])
                    nc.scalar.dma_start(out=x_t[:, 0, 2048:], in_=x_ext[s, 0, :, 2048:])
                    nc.sync.dma_start(out=x_t[:, 1, 0:2048], in_=x_ext[s, 1, :, 0:2048])
                    nc.scalar.dma_start(out=x_t[:, 1, 2048:], in_=x_ext[s, 1, :, 2048:])
                else:
                    nc.sync.dma_start(
                        out=x_t, in_=x_ext[s].rearrange("t p f -> p t f")
                    )
                ysum = small.tile([128, 2], f32)
                for t in range(2):
                    nc.scalar.activation(
                        out=gapjunk,
                        in_=x_t[:, t],
                        func=AF.Copy,
                        bias=0.0,
                        scale=1.0,
                        accum_out=ysum[:, t : t + 1],
                    )
                st[s].update(x_t=x_t, ysum=ysum)

            def stage_E(s):
                x_t, ysum = st[s]["x_t"], st[s]["ysum"]
                py_t = ps_y.tile([128, 2], f32)
                for mt in range(2):
                    for kt in range(2):
                        nc.tensor.matmul(
                            py_t[:, mt : mt + 1],
                            lhsT=bmat_sb[:, kt, mt * 128 : (mt + 1) * 128],
                            rhs=ysum[:, kt : kt + 1],
                            start=(kt == 0),
                            stop=(kt == 1),
                        )
                yscale = small.tile([128, 2], f32)
                nc.scalar.copy(out=yscale, in_=py_t)
                yc = pyc.tile([128, 2, HW], bf16)
                for t in range(2):
                    nc.scalar.activation(
                        out=yc[:, t],
                        in_=x_t[:, t],
                        func=AF.Sigmoid,
                        scale=yscale[:, t : t + 1],
                    )
                st[s].update(yc=yc)

            def stage_M(s):
                pbuf = s % 2
                yc = st[s]["yc"]
                pm = ppm.tile([128, HW], bf16)
                # mean over channels -> psum rows {0,32,64,96}
                pm_mean = ps_mean.tile([128, 1024], f32)
                for n in range(8):
                    row = 32 * (n // 2)
                    col = 512 * (n % 2)
                    for t in range(2):
                        nc.tensor.matmul(
                            pm_mean[row : row + 1, col : col + 512],
                            lhsT=ones_bf,
                            rhs=yc[:, t, 512 * n : 512 * (n + 1)],
                            start=(t == 0),
                            stop=(t == 1),
                            tile_position=(0, row),
                        )
                mean_row = small.tile([128, 1024], bf16)
                nc.scalar.copy(out=mean_row, in_=pm_mean)
                for k in range(4):
                    nc.scalar.dma_start(
                        out=pads[pbuf, 0, 2 + 16 * k : 2 + 16 * (k + 1), 2:66],
                        in_=mean_row[32 * k : 32 * k + 1, :].rearrange(
                            "p (hh w) -> p hh w", hh=16
                        ),
                    )

                # max over channels
                nc.vector.tensor_max(out=pm, in0=yc[:, 0], in1=yc[:, 1])
                maxseg = small.tile([128, 32], bf16)
                for g in range(4):
                    pt_t = ps_t.tile([128, 8, 128], bf16, tag="pt")
                    for b in range(8):
                        bb = 8 * g + b
                        nc.tensor.transpose(
                            pt_t[:, b], pm[:, 128 * bb : 128 * (bb + 1)], ident
                        )
                    nc.vector.tensor_reduce(
                        out=maxseg[:, 8 * g : 8 * (g + 1)],
                        in_=pt_t,
                        axis=AX.X,
                        op=ALU.max,
                    )
                pms_t = ps_t.tile([32, 128], bf16, tag="pt")
                nc.tensor.transpose(pms_t, maxseg, ident)
                maxrow = small.tile([32, 128], bf16)
                nc.scalar.copy(out=maxrow, in_=pms_t)
                nc.scalar.dma_start(
                    out=pads[pbuf, 1, 2:66, 2:66].rearrange(
                        "(b hh) w -> b hh w", b=32
                    ),
                    in_=maxrow.rearrange("b (hh w) -> b hh w", hh=2),
                )

                # 68-pitch patches; window length 4348, last 4 cols stale
                patches = ppatch.tile([50, PJ4], bf16)
                eng = (nc.gpsimd, nc.scalar, nc.sync)
                for c in range(2):
                    for ky in range(5):
                        src = bass.AP(
                            tensor=pads,
                            offset=(pbuf * 2 + c) * PAD * PAD + ky * PAD,
                            ap=[[1, 5], [1, PJ4 - 4]],
                        )
                        r0 = 25 * c + 5 * ky
                        eng[(c * 5 + ky) % 3].dma_start(
                            out=patches[r0 : r0 + 5, 0 : PJ4 - 4], in_=src
                        )

                # 5x5 conv as matmul + sigmoid -> sa rows {0,32,64,96}
                pm_sa = ps_sa.tile([128, PJ], f32)
                for k in range(4):
                    for c0, nn in ((0, 512), (512, 512), (1024, PJ - 1024)):
                        nc.tensor.matmul(
                            pm_sa[32 * k : 32 * k + 1, c0 : c0 + nn],
                            lhsT=ws_bf,
                            rhs=patches[:, PJ * k + c0 : PJ * k + c0 + nn],
                            start=True,
                            stop=True,
                            tile_position=(0, 32 * k),
                        )
                sa_row = small.tile([128, PJ], bf16)
                nc.scalar.activation(out=sa_row, in_=pm_sa, func=AF.Sigmoid)

                # sa (68-pitch rows {0,32,64,96}) -> pixel-linear DRAM,
                # broadcast back per 1024-pixel chunk on alternating engines
                sab = psab.tile([128, HW], bf16)
                for k in range(4):
                    nc.sync.dma_start(
                        out=sa_dram[pbuf, 1024 * k : 1024 * (k + 1)].rearrange(
                            "(p hh w) -> p hh w", p=1, hh=16
                        ),
                        in_=sa_row[32 * k : 32 * k + 1, :].rearrange(
                            "p (hh w) -> p hh w", hh=16
                        )[:, :, 0:64],
                    )
                src_b = bass.AP(
                    tensor=sa_dram,
                    offset=pbuf * HW,
                    ap=[[0, 128], [1, HW]],
                )
                nc.gpsimd.dma_start(out=sab, in_=src_b)
                st[s].update(sab=sab)

            def stage_T(s):
                x_t, yc, sab = st[s]["x_t"], st[s]["yc"], st[s]["sab"]
                if s == SPC - 1:
                    for k in range(4):
                        c0, c1 = 1024 * k, 1024 * (k + 1)
                        sc = sab[:, c0:c1]
                        sc2 = bass.AP(
                            tensor=sc.tensor, offset=sc.offset,
                            ap=[list(sc.ap[0]), [0, 2], list(sc.ap[1])],
                        )
                        nc.vector.tensor_mul(
                            out=yc[:, :, c0:c1], in0=yc[:, :, c0:c1], in1=sc2,
                        )
                        nc.vector.tensor_add(
                            out=x_t[:, :, c0:c1], in0=yc[:, :, c0:c1],
                            in1=x_t[:, :, c0:c1],
                        )
                    nc.gpsimd.dma_start(out=out_ext[s, 0], in_=x_t[:, 0])
                    nc.sync.dma_start(out=out_ext[s, 1], in_=x_t[:, 1])
                else:
                    for t in range(2):
                        nc.vector.tensor_mul(out=yc[:, t], in0=yc[:, t], in1=sab)
                        nc.vector.tensor_add(out=x_t[:, t], in0=yc[:, t], in1=x_t[:, t])
                    nc.gpsimd.dma_start(
                        out=out_ext[s].rearrange("t p f -> p t f"), in_=x_t
                    )

            for step in range(SPC + 3):
                for d, fn in ((1, stage_E), (0, stage_H), (3, stage_T), (2, stage_M)):
                    s = step - d
                    if 0 <= s < SPC:
                        fn(s)

    nc.compile()
    return nc


def _prep_inputs(x, w_c, w_s):
    """Shard + build per-core input maps (host side, cheap)."""
    import ml_dtypes

    wc = np.asarray(w_c, dtype=np.float32).reshape(5)
    ws4 = np.asarray(w_s, dtype=np.float32).reshape(2, 5, 5)

    # banded matrix: y'[m] = sum_k y[k] * wc[k - m + 2];  GAP 1/4096 folded in
    k = np.arange(C)[:, None]
    m = np.arange(C)[None, :]
    d = k - m + 2
    bmat = np.where((d >= 0) & (d < 5), wc[np.clip(d, 0, 4)], 0.0).astype(np.float32)
    bmat = (bmat / HW).reshape(2, 128, 256)

    # conv weights vector, rows = c*25 + ky*5 + kx ; channel-mean 1/256 folded in
    wsv = ws4.copy()
    wsv[0] /= C
    wsv = wsv.reshape(50, 1).astype(np.float32)

    xs = np.asarray(x, dtype=np.float32).astype(ml_dtypes.bfloat16).reshape(
        N_CORES, SPC, 2, 128, HW
    )
    in_maps = [
        {"x": xs[i], "bmat": bmat, "ws": wsv} for i in range(N_CORES)
    ]
    return in_maps


def run(x, w_c, w_s, trace=False):
    from concourse.bass_utils import run_bass_kernel_spmd

    if "nc" not in _cache:
        _cache["nc"] = _build_graph()
    nc = _cache["nc"]
    in_maps = _prep_inputs(x, w_c, w_s)
    res = run_bass_kernel_spmd(
        nc, in_maps, core_ids=list(range(N_CORES)), trace=trace
    )
    out = np.concatenate(
        [
            res.results[i]["out"].astype(np.float32).reshape(SPC, C, H, W)
            for i in range(N_CORES)
        ],
        axis=0,
    )
    return out, res


def kernel(x, w_c, w_s):
    out, _ = run(x, w_c, w_s, trace=False)
    return out.astype(np.float32)


# revision 34
# speedup vs baseline: 1.1473x; 1.1473x over previous
"""Trainium2 Bass kernel for Adapt_CSA (ECA channel attention + CBAM spatial attention).

Reference computation (per sample, x: (C=256, H=64, W=64) f32):
  y    = mean(x, (H,W))                       # (C,)
  y'   = conv1d(y, w_c, SAME, k=5)            # (C,)
  yc   = sigmoid(x * y'[:, None, None])       # (C, H, W)
  avg  = mean(yc, C); mx = max(yc, C)         # (H, W) each
  sa   = sigmoid(conv2d([avg, mx], w_s, SAME))# (1, H, W)
  out  = yc * sa + x

Data parallel over batch: 32 samples -> 8 cores x 4 samples.
Layout on chip: channels on partitions (2 tiles of (128, 4096) per sample).
x is uploaded in bf16 (rel-err budget is 2e-2); output stored bf16 and
widened to f32 on the host.

Emission is software-pipelined in 4 skewed stages per sample --
H: load + global-avg-pool, E: channel conv1d + sigmoid, M: spatial branch,
T: multiply + residual + store -- so that every engine's (in-order)
instruction stream is phase-sorted and sample s+1's head never queues
behind sample s's tail on the same engine.

The 5x5 conv uses "68-pitch" patches: avg/max maps are written into
zero-padded 68x68 DRAM images; each of the 50 patch rows (c, ky, kx) is one
contiguous 4348-element window of the padded image, so the conv becomes a
single (50 x 4352) matmul against the 50 folded weights and the output is
pixel-indexed with pitch 68 (columns 64..67 of each row are garbage).
"""

import os
import sys

import numpy as np

sys.path.insert(0, "/opt/trn_rl_repo")

B, C, H, W = 32, 256, 64, 64
HW = H * W  # 4096
N_CORES = 8
SPC = B // N_CORES  # samples per core = 4
PAD = 68  # 64 + 2*2 zero border for SAME 5x5 conv
PJ = 16 * PAD  # 1088: 68-pitch pixel range per 16-row block
PJ4 = 4 * PJ  # 4352

_cache = {}


def _build_graph():
    import concourse.bass as bass
    import concourse.bacc as bacc
    import concourse.tile as tile
    from concourse import masks, mybir

    f32 = mybir.dt.float32
    bf16 = mybir.dt.bfloat16
    AF = mybir.ActivationFunctionType
    ALU = mybir.AluOpType
    AX = mybir.AxisListType

    nc = bacc.Bacc("TRN2", target_bir_lowering=False)

    x_ext = nc.declare_dram_parameter("x", [SPC, 2, 128, HW], bf16, isOutput=False)
    bmat_ext = nc.declare_dram_parameter("bmat", [2, 128, 256], f32, isOutput=False)
    ws_ext = nc.declare_dram_parameter("ws", [50, 1], f32, isOutput=False)
    out_ext = nc.declare_dram_parameter("out", [SPC, 2, 128, HW], bf16, isOutput=True)

    # double-buffered padded avg/max maps in DRAM: [buf][c][ph][pw]
    pads = nc.dram_tensor("pads", [2, 2, PAD, PAD], bf16)
    # double-buffered sa row (pixel-linear) in DRAM for partition broadcast
    sa_dram = nc.dram_tensor("sa_dram", [2, HW], bf16)

    with tile.TileContext(nc) as tc:
        with (
            tc.tile_pool(name="singles", bufs=1) as singles,
            tc.tile_pool(name="px", bufs=4) as px,
            tc.tile_pool(name="pyc", bufs=4) as pyc,
            tc.tile_pool(name="ppm", bufs=3) as ppm,
            tc.tile_pool(name="ppatch", bufs=2) as ppatch,
            tc.tile_pool(name="psab", bufs=2) as psab,
            tc.tile_pool(name="small", bufs=4) as small,
            tc.tile_pool(name="ps_y", bufs=1, space="PSUM") as ps_y,
            tc.tile_pool(name="ps_t", bufs=2, space="PSUM") as ps_t,
            tc.tile_pool(name="ps_mean", bufs=1, space="PSUM") as ps_mean,
            tc.tile_pool(name="ps_sa", bufs=1, space="PSUM") as ps_sa,
        ):
            # ---- constants ----
            bmat_sb = singles.tile([128, 2, 256], f32)
            nc.sync.dma_start(out=bmat_sb, in_=bmat_ext[:].rearrange("t p m -> p t m"))
            ws_f32 = singles.tile([50, 1], f32)
            nc.sync.dma_start(out=ws_f32, in_=ws_ext[:])
            ws_bf = singles.tile([50, 1], bf16)
            nc.vector.tensor_copy(out=ws_bf, in_=ws_f32)
            ident = singles.tile([128, 128], bf16)
            masks.make_identity(nc, ident[:])
            ones_bf = singles.tile([128, 1], bf16)
            nc.vector.memset(ones_bf, 1.0)
            zero68 = singles.tile([PAD, 2, PAD], bf16)
            nc.vector.memset(zero68, 0.0)
            for j in range(2):
                nc.sync.dma_start(
                    out=pads[j].rearrange("c h w -> h c w"), in_=zero68
                )

            # per-sample state carried between stages
            st = [dict() for _ in range(SPC)]

            def stage_H(s):
                x_t = px.tile([128, 2, HW], bf16)
                nc.sync.dma_start(
                    out=x_t, in_=x_ext[s].rearrange("t p f -> p t f")
                )
                # GAP sums; the Copy output is scratch (dumped into the
                # pairmax tile, overwritten later in stage M)
                pm = ppm.tile([128, HW], bf16)
                ysum = small.tile([128, 2], f32)
                for t in range(2):
                    nc.scalar.activation(
                        out=pm,
                        in_=x_t[:, t],
                        func=AF.Copy,
                        bias=0.0,
                        scale=1.0,
                        accum_out=ysum[:, t : t + 1],
                    )
                st[s].update(x_t=x_t, pm=pm, ysum=ysum)

            def stage_E(s):
                x_t, ysum = st[s]["x_t"], st[s]["ysum"]
                py_t = ps_y.tile([128, 2], f32)
                for mt in range(2):
                    for kt in range(2):
                        nc.tensor.matmul(
                            py_t[:, mt : mt + 1],
                            lhsT=bmat_sb[:, kt, mt * 128 : (mt + 1) * 128],
                            rhs=ysum[:, kt : kt + 1],
                            start=(kt == 0),
                            stop=(kt == 1),
                        )
                yscale = small.tile([128, 2], f32)
                nc.scalar.copy(out=yscale, in_=py_t)
                yc = pyc.tile([128, 2, HW], bf16)
                for t in range(2):
                    nc.scalar.activation(
                        out=yc[:, t],
                        in_=x_t[:, t],
                        func=AF.Sigmoid,
                        scale=yscale[:, t : t + 1],
                    )
                st[s].update(yc=yc)

            def stage_M(s):
                pbuf = s % 2
                yc, pm = st[s]["yc"], st[s]["pm"]
                # mean over channels -> psum rows {0,32,64,96}
                pm_mean = ps_mean.tile([128, 1024], f32)
                for n in range(8):
                    row = 32 * (n // 2)
                    col = 512 * (n % 2)
                    for t in range(2):
                        nc.tensor.matmul(
                            pm_mean[row : row + 1, col : col + 512],
                            lhsT=ones_bf,
                            rhs=yc[:, t, 512 * n : 512 * (n + 1)],
                            start=(t == 0),
                            stop=(t == 1),
                            tile_position=(0, row),
                        )
                mean_row = small.tile([128, 1024], bf16)
                nc.scalar.copy(out=mean_row, in_=pm_mean)
                for k in range(4):
                    nc.scalar.dma_start(
                        out=pads[pbuf, 0, 2 + 16 * k : 2 + 16 * (k + 1), 2:66],
                        in_=mean_row[32 * k : 32 * k + 1, :].rearrange(
                            "p (hh w) -> p hh w", hh=16
                        ),
                    )

                # max over channels
                nc.vector.tensor_max(out=pm, in0=yc[:, 0], in1=yc[:, 1])
                maxseg = small.tile([128, 32], bf16)
                for g in range(4):
                    pt_t = ps_t.tile([128, 8, 128], bf16, tag="pt")
                    for b in range(8):
                        bb = 8 * g + b
                        nc.tensor.transpose(
                            pt_t[:, b], pm[:, 128 * bb : 128 * (bb + 1)], ident
                        )
                    nc.vector.tensor_reduce(
                        out=maxseg[:, 8 * g : 8 * (g + 1)],
                        in_=pt_t,
                        axis=AX.X,
                        op=ALU.max,
                    )
                pms_t = ps_t.tile([32, 128], bf16, tag="pt")
                nc.tensor.transpose(pms_t, maxseg, ident)
                maxrow = small.tile([32, 128], bf16)
                nc.scalar.copy(out=maxrow, in_=pms_t)
                nc.scalar.dma_start(
                    out=pads[pbuf, 1, 2:66, 2:66].rearrange(
                        "(b hh) w -> b hh w", b=32
                    ),
                    in_=maxrow.rearrange("b (hh w) -> b hh w", hh=2),
                )

                # 68-pitch patches; window length 4348, last 4 cols stale
                patches = ppatch.tile([50, PJ4], bf16)
                eng = (nc.gpsimd, nc.scalar, nc.sync)
                for c in range(2):
                    for ky in range(5):
                        src = bass.AP(
                            tensor=pads,
                            offset=(pbuf * 2 + c) * PAD * PAD + ky * PAD,
                            ap=[[1, 5], [1, PJ4 - 4]],
                        )
                        r0 = 25 * c + 5 * ky
                        eng[(c * 5 + ky) % 3].dma_start(
                            out=patches[r0 : r0 + 5, 0 : PJ4 - 4], in_=src
                        )

                # 5x5 conv as matmul + sigmoid -> sa rows {0,32,64,96}
                pm_sa = ps_sa.tile([128, PJ], f32)
                for k in range(4):
                    for c0, nn in ((0, 512), (512, 512), (1024, PJ - 1024)):
                        nc.tensor.matmul(
                            pm_sa[32 * k : 32 * k + 1, c0 : c0 + nn],
                            lhsT=ws_bf,
                            rhs=patches[:, PJ * k + c0 : PJ * k + c0 + nn],
                            start=True,
                            stop=True,
                            tile_position=(0, 32 * k),
                        )
                sa_row = small.tile([128, PJ], bf16)
                nc.scalar.activation(out=sa_row, in_=pm_sa, func=AF.Sigmoid)

                # sa (68-pitch rows {0,32,64,96}) -> pixel-linear DRAM,
                # broadcast back per 1024-pixel chunk on alternating engines
                sab = psab.tile([128, HW], bf16)
                for k in range(4):
                    nc.sync.dma_start(
                        out=sa_dram[pbuf, 1024 * k : 1024 * (k + 1)].rearrange(
                            "(p hh w) -> p hh w", p=1, hh=16
                        ),
                        in_=sa_row[32 * k : 32 * k + 1, :].rearrange(
                            "p (hh w) -> p hh w", hh=16
                        )[:, :, 0:64],
                    )
                src_b = bass.AP(
                    tensor=sa_dram,
                    offset=pbuf * HW,
                    ap=[[0, 128], [1, HW]],
                )
                nc.gpsimd.dma_start(out=sab, in_=src_b)
                st[s].update(sab=sab)

            def stage_T(s):
                x_t, yc, sab = st[s]["x_t"], st[s]["yc"], st[s]["sab"]
                for t in range(2):
                    nc.vector.tensor_mul(out=yc[:, t], in0=yc[:, t], in1=sab)
                    nc.vector.tensor_add(out=x_t[:, t], in0=yc[:, t], in1=x_t[:, t])
                nc.gpsimd.dma_start(
                    out=out_ext[s].rearrange("t p f -> p t f"), in_=x_t
                )

            for step in range(SPC + 3):
                for d, fn in ((1, stage_E), (0, stage_H), (3, stage_T), (2, stage_M)):
                    s = step - d
                    if 0 <= s < SPC:
                        fn(s)

    nc.compile()
    return nc


def _prep_inputs(x, w_c, w_s):
    """Shard + build per-core input maps (host side, cheap)."""
    import ml_dtypes

    wc = np.asarray(w_c, dtype=np.float32).reshape(5)
    ws4 = np.asarray(w_s, dtype=np.float32).reshape(2, 5, 5)

    # banded matrix: y'[m] = sum_k y[k] * wc[k - m + 2];  GAP 1/4096 folded in
    k = np.arange(C)[:, None]
    m = np.arange(C)[None, :]
    d = k - m + 2
    bmat = np.where((d >= 0) & (d < 5), wc[np.clip(d, 0, 4)], 0.0).astype(np.float32)
    bmat = (bmat / HW).reshape(2, 128, 256)

    # conv weights vector, rows = c*25 + ky*5 + kx ; channel-mean 1/256 folded in
    wsv = ws4.copy()
    wsv[0] /= C
    wsv = wsv.reshape(50, 1).astype(np.float32)

    xs = np.asarray(x, dtype=np.float32).astype(ml_dtypes.bfloat16).reshape(
        N_CORES, SPC, 2, 128, HW
    )
    in_maps = [
        {"x": xs[i], "bmat": bmat, "ws": wsv} for i in range(N_CORES)
    ]
    return in_maps


def run(x, w_c, w_s, trace=False):
    from concourse.bass_utils import run_bass_kernel_spmd

    if "nc" not in _cache:
        _cache["nc"] = _build_graph()
    nc = _cache["nc"]
    in_maps = _prep_inputs(x, w_c, w_s)
    res = run_bass_kernel_spmd(
        nc, in_maps, core_ids=list(range(N_CORES)), trace=trace
    )
    out = np.concatenate(
        [
            res.results[i]["out"].astype(np.float32).reshape(SPC, C, H, W)
            for i in range(N_CORES)
        ],
        axis=0,
    )
    return out, res


def kernel(x, w_c, w_s):
    out, _ = run(x, w_c, w_s, trace=False)
    return out.astype(np.float32)
